# revision 16
# baseline (speedup 1.0000x reference)
"""Trainium2 Bass kernel for nn_AttentionLayer (segment softmax attention pooling).

Computation (reference):
    h = tanh(x @ W1 + b1)            # [N, A]
    s = h @ W2 + b2                  # [N, 1]
    per-segment softmax over s, out[b] = sum_i softmax_w_i * x_i   # [B, D]

Strategy (v3, ACT-saturated pipeline + fp8 e-broadcast):
  - Shard the N=500k instances across 8 NeuronCores (data parallel), weights
    replicated.  Host pre-converts x to bf16 and pre-transposes so each core
    streams xT [D=128, rows] tiles.
  - Per core, one pass over x, 2048-row chunks.  Per chunk:
      PE : hT = W1^T @ xT (4x 512-col matmuls, bf16)
      ACT: th = tanh(hT + b1) -> bf16, two 1024-col halves
      PE : 4 col-tiled score matmuls -> sg [128, 512] grouped scores
           (group g = rows 512g..512g+511 on partitions 32g..32g+31)
      ACT: e = exp(sg + b2) -> float8e4 [128, 512] grouped (fp8 is safe:
           scores are O(+-4), exp <= ~50 << 448; the ~4% fp8 noise averages
           out over ~2k instances per segment)
      PE : 4x DoubleRow fp8 ones-matmuls broadcast e rows -> ebc [128, 2048]
           f32 in PSUM (fp8 DoubleRow streams at 2x bf16, so the broadcast
           costs half the baseline's bf16 ones-matmuls)
      DVE: affine_mul_reduce(xT * ebc) summed per 1024-row window -> wacc
  - The steady-state period is the ACT engine (2x tanh + 1x exp ~= 2.9us).
    score(c) is emitted PE-last of its iteration and exp(c) ACT-last of the
    NEXT iteration, so exp never waits on score inside a period (the
    baseline lost ~0.55us/period to that gap).  ebc(c)+windows(c) run two
    iterations behind.  With this order the baseline 8-bank PSUM layout
    (h halves | sg aliased into ebc bank 0) stays race-free.
  - e blocks and wacc columns are DMA'd out incrementally per chunk, so the
    kernel tail is just the pipeline drain.
  - Host: denominators via bincount over the exported fp8 e; pure windows
    straight from wacc; windows containing a segment boundary recomputed on
    the host from x and e.  exp without max subtraction is safe (scores
    O(+-4)); numerator and denominator share the same fp8 e bytes.
"""

import numpy as np

# Problem constants (hardcoded per contract; kernel.py must be self-contained).
N = 500_000
D = 128
A = 128
B = 256
NCORES = 8
RPC = N // NCORES            # rows per core = 62500
CHUNK = 2048                 # rows per streamed tile
WIN = 1024                   # rows per reduction window
G = CHUNK // 4               # grouped score tile free dim = 512
NCHUNK = -(-RPC // CHUNK)    # 31
RPAD = NCHUNK * CHUNK        # 63488
NWIN = RPAD // WIN           # 62
MM_N = 512                   # PE moving-operand max free dim
EBLK = NCHUNK + 1            # e blocks incl. one junk tail block

_prog_cache = {}


def _build_program():
    import concourse.bacc as bacc
    from concourse import mybir
    from concourse.tile import TileContext

    f32 = mybir.dt.float32
    bf16 = mybir.dt.bfloat16
    fp8 = mybir.dt.float8e4
    u8 = mybir.dt.uint8
    DR = mybir.MatmulPerfMode.DoubleRow
    Act = mybir.ActivationFunctionType
    nc = bacc.Bacc("TRN2", target_bir_lowering=False, debug=False,
                   num_devices=NCORES)

    xt = nc.dram_tensor("xt", [D, RPAD], bf16, kind="ExternalInput")
    w1 = nc.dram_tensor("w1", [D, A], bf16, kind="ExternalInput")
    w2r = nc.dram_tensor("w2r", [A, 128], bf16, kind="ExternalInput")
    # DoubleRow lhsT pairs: [:, 0:128] = 1 (k=0 slot), [:, 128:256] = 0.
    ones2 = nc.dram_tensor("ones2", [128, 256], fp8, kind="ExternalInput")
    b1 = nc.dram_tensor("b1", [A, 1], f32, kind="ExternalInput")
    b2 = nc.dram_tensor("b2", [128, 1], f32, kind="ExternalInput")
    wacc = nc.dram_tensor("wacc", [D, NWIN], f32, kind="ExternalOutput")
    eout = nc.dram_tensor("eout", [128, NCHUNK * G], u8, kind="ExternalOutput")

    with TileContext(nc) as tc:
        with tc.tile_pool(name="const", bufs=1) as cpool, \
             tc.tile_pool(name="xtp", bufs=6) as xpool, \
             tc.tile_pool(name="thp", bufs=2) as thpool, \
             tc.tile_pool(name="junkp", bufs=2) as jpool, \
             tc.tile_pool(name="accp", bufs=1) as apool, \
             tc.tile_pool(name="psb", bufs=1, space="PSUM") as psb:

            w1sb = cpool.tile([D, A], bf16, tag="w1")
            w2rsb = cpool.tile([A, 128], bf16, tag="w2r")
            onesb = cpool.tile([128, 256], fp8, tag="ones2")
            b1sb = cpool.tile([A, 1], f32, tag="b1")
            b2sb = cpool.tile([128, 1], f32, tag="b2")
            # Junk-matmul input + dummy-activation scratch (disjoint cols).
            warm = cpool.tile([128, MM_N + 8], bf16, tag="warm")

            waccsb = apool.tile([D, NWIN], f32, tag="wacc")
            # All chunks' grouped e tiles as fp8 bytes (16 KB/partition),
            # plus one junk tail block for the last chunk's DoubleRow pair.
            eall = apool.tile([128, EBLK * G], u8, tag="eall")

            # PSUM: h halves banks 0-3 | ebc banks 4-7 | sg aliased under
            # ebc g3 (bank 7), so the exp->ebc WAR only delays g3, and the
            # broadcast for g0-g2 (the big 1536-row window) free-runs.
            pbig = psb.tile([128, 2 * CHUNK], f32, tag="pbig")
            HALF = CHUNK // 2
            hregs = [pbig[:, 0:HALF], pbig[:, HALF:CHUNK]]
            sgreg = pbig[:, 2 * CHUNK - G:2 * CHUNK]
            ebc = pbig[:, CHUNK:2 * CHUNK]

            xtiles, ths = {}, {}

            def load_chunk(c, split=False):
                xtile = xpool.tile([D, CHUNK], bf16, tag="x")
                base = c * CHUNK
                if split:
                    # Quarter loads issued from different engines so the
                    # first h-matmul unblocks after 128 KB, not 512 KB.
                    engines = [nc.sync, nc.scalar, nc.gpsimd, nc.sync]
                    for q, eng in enumerate(engines):
                        eng.dma_start(
                            out=xtile[:, q * MM_N:(q + 1) * MM_N],
                            in_=xt[:, base + q * MM_N:base + (q + 1) * MM_N])
                else:
                    nc.gpsimd.dma_start(out=xtile[:],
                                        in_=xt[:, base:base + CHUNK])
                xtiles[c] = xtile

            # ---- startup: spread issue work across the DMA-capable engines
            # (sync/SP, scalar/Activation, gpsimd) ----
            nc.vector.memset(warm[:], 0.0)
            load_chunk(0, split=True)
            nc.scalar.dma_start(out=w1sb[:], in_=w1[:])
            nc.sync.dma_start(out=b1sb[:], in_=b1[:])
            nc.sync.dma_start(out=b2sb[:], in_=b2[:])
            nc.sync.dma_start(out=w2rsb[:], in_=w2r[:])

            # Warm the PE clock gate with junk matmuls on the zeroed warm
            # tile (output overwritten by h(0), which also sets start=True).
            for _ in range(2):
                nc.tensor.matmul(out=hregs[0][:, 0:MM_N],
                                 lhsT=warm[:, 0:128], rhs=warm[:, 0:MM_N],
                                 start=True, stop=True)
            # Load the tanh/exp ACT table during the initial DMA wait.
            nc.scalar.activation(out=warm[:, MM_N:MM_N + 1],
                                 in_=warm[:, MM_N + 2:MM_N + 3],
                                 func=Act.Tanh)
            nc.scalar.activation(out=warm[:, MM_N + 1:MM_N + 2],
                                 in_=warm[:, MM_N + 2:MM_N + 3],
                                 func=Act.Exp)
            nc.scalar.dma_start(out=onesb[:], in_=ones2[:])

            def stage_h(c):
                xtile = xtiles[c]
                th = thpool.tile([A, CHUNK], bf16, tag="th")
                ths[c] = th
                for hh in range(2):
                    off = hh * HALF
                    for i in range(HALF // MM_N):
                        nc.tensor.matmul(
                            out=hregs[hh][:, i * MM_N:(i + 1) * MM_N],
                            lhsT=w1sb[:],
                            rhs=xtile[:, off + i * MM_N:off + (i + 1) * MM_N],
                            start=True, stop=True)
                    nc.scalar.activation(
                        out=th[:, off:off + HALF],
                        in_=hregs[hh],
                        func=Act.Tanh,
                        bias=b1sb[:, 0:1])

            def stage_score(c):
                th = ths.pop(c)
                for g in range(4):
                    nc.tensor.matmul(
                        out=sgreg[32 * g:32 * (g + 1), :],
                        lhsT=w2rsb[:, 32 * g:32 * (g + 1)],
                        rhs=th[:, G * g:G * (g + 1)],
                        start=True, stop=True,
                        tile_position=(0, 32 * g))

            def stage_exp(c):
                nc.scalar.activation(
                    out=eall[:, c * G:(c + 1) * G].bitcast(fp8),
                    in_=sgreg,
                    func=Act.Exp,
                    bias=b2sb[:, 0:1])

            def stage_ebc(c):
                # Broadcast e group rows to all 128 partitions via fp8
                # DoubleRow ones-matmuls (k=1 slot of lhsT is zero, so the
                # adjacent e block read as the k=1 operand is inert).
                for g in range(4):
                    rhs = eall[32 * g:32 * g + 1,
                               c * G:(c + 2) * G].bitcast(fp8).rearrange(
                                   "p (k n) -> p k n", k=2)
                    lhsT = onesb[32 * g:32 * g + 1, :].rearrange(
                        "p (k m) -> p k m", k=2)
                    nc.tensor.matmul(
                        out=ebc[:, G * g:G * (g + 1)],
                        lhsT=lhsT, rhs=rhs,
                        start=True, stop=True,
                        perf_mode=DR,
                        tile_position=(32 * g, 0))

            def stage_windows(c):
                # Uneven (1536, 512) windows: w1 covers exactly ebc bank 7
                # (the sg alias), so the next score's WAR wait is on the
                # short window's read, and w0 never touches the alias.
                xtile_c = xtiles.pop(c)
                for w, (lo, hi) in enumerate(((0, CHUNK - G), (CHUNK - G, CHUNK))):
                    junk = jpool.tile([D, 1], f32, tag="junk")
                    nc.vector.affine_mul_reduce(
                        out=junk[:].to_broadcast([D, hi - lo]),
                        accum_out=waccsb[:, 2 * c + w:2 * c + w + 1],
                        in0=xtile_c[:, lo:hi],
                        in1=ebc[:, lo:hi],
                        scale=1.0,
                        bias=0.0)

            def stage_export(c):
                nc.sync.dma_start(out=eout[:, c * G:(c + 1) * G],
                                  in_=eall[:, c * G:(c + 1) * G])
                nc.sync.dma_start(out=wacc[:, 2 * c:2 * c + 2],
                                  in_=waccsb[:, 2 * c:2 * c + 2])

            # ---- prologue (chunk 0 already loading since startup) ----
            load_chunk(1)
            load_chunk(2)
            # Junk-block init (and defined first-sweep k=1 reads) for the
            # DoubleRow pairs; overlaps the initial DMA wait on the Pool
            # engine.
            nc.gpsimd.memset(eall[:].bitcast(f32), 0.0)
            stage_h(0)
            load_chunk(3)
            stage_h(1)
            stage_score(0)
            load_chunk(4)
            stage_exp(0)
            stage_h(2)
            stage_score(1)
            # ---- steady state ----
            # Skewed pipeline; per iteration c the engines see, in order:
            #   ACT: exp(c-2) [its score finished a full period ago],
            #        tanh halves (c)
            #   PE : h(c) [4], ebc(c-3) [4], score(c-1) [4]
            #   DVE: w0(c-3) [512 rows = the sg-aliased bank], w1(c-3)
            # The WAR chain on the sg/ebc-g0 bank is score(x) -> exp(x) ->
            # ebc(x-1) -> w0(x-1) -> score(x+1), with each hop landing in a
            # later iteration, so the framework's reader+1 WAR waits never
            # pull next-period ACT work into the cycle.
            for c in range(3, NCHUNK):
                if c + 2 < NCHUNK:
                    load_chunk(c + 2)
                stage_exp(c - 2)
                stage_h(c)
                stage_ebc(c - 3)
                stage_windows(c - 3)
                stage_score(c - 1)
                stage_export(c - 3)
            # ---- drain ----
            stage_exp(NCHUNK - 2)
            stage_ebc(NCHUNK - 3)
            stage_windows(NCHUNK - 3)
            stage_score(NCHUNK - 1)
            stage_export(NCHUNK - 3)
            stage_exp(NCHUNK - 1)
            stage_ebc(NCHUNK - 2)
            stage_windows(NCHUNK - 2)
            stage_export(NCHUNK - 2)
            stage_ebc(NCHUNK - 1)
            stage_windows(NCHUNK - 1)
            stage_export(NCHUNK - 1)

    nc.compile()
    return nc


def _to_bf16(a):
    import ml_dtypes
    return np.asarray(a).astype(ml_dtypes.bfloat16)


def _run_device(xt_shards, W1, W2, b1, b2, trace=False):
    from concourse.bass_utils import run_bass_kernel_spmd
    import ml_dtypes

    if "prog" not in _prog_cache:
        _prog_cache["prog"] = _build_program()
    nc = _prog_cache["prog"]

    w1_in = np.ascontiguousarray(_to_bf16(W1))
    w2r_in = np.ascontiguousarray(
        _to_bf16(np.tile(np.asarray(W2, dtype=np.float32).reshape(A, 1),
                         (1, 128))))
    ones2_in = np.zeros((128, 256), dtype=ml_dtypes.float8_e4m3)
    ones2_in[:, 0:128] = 1.0
    b1_in = np.ascontiguousarray(np.asarray(b1, np.float32).reshape(A, 1))
    b2_in = np.full((128, 1), np.float32(b2), dtype=np.float32)

    in_maps = [{"xt": xt_shards[i], "w1": w1_in, "w2r": w2r_in,
                "ones2": ones2_in, "b1": b1_in, "b2": b2_in}
               for i in range(NCORES)]
    res = run_bass_kernel_spmd(nc, in_maps, core_ids=list(range(NCORES)),
                               trace=trace)
    return res


def kernel(x, batch_index, W1, b1, W2, b2, _want_results=False, _trace=False):
    import ml_dtypes

    x = np.asarray(x, dtype=np.float32)
    bi64 = np.asarray(batch_index).astype(np.int64)
    b2v = float(np.asarray(b2, dtype=np.float32).reshape(-1)[0])

    assert x.shape == (N, D)

    # Host pre-transpose + bf16: xT [D, N] bf16, then per-core padded shards.
    xb = x.astype(ml_dtypes.bfloat16)
    xtb = xb.T
    xt_shards = []
    for i in range(NCORES):
        sh = np.zeros((D, RPAD), dtype=ml_dtypes.bfloat16)
        sh[:, :RPC] = xtb[:, i * RPC:(i + 1) * RPC]
        xt_shards.append(np.ascontiguousarray(sh))

    res = _run_device(xt_shards, W1, W2, b1, b2v, trace=_trace)

    # Gather device outputs.
    e = np.empty(N, dtype=np.float32)
    waccs = []
    for i in range(NCORES):
        eb = res.results[i]["eout"]  # uint8 fp8e4m3 bytes [128, NCHUNK*G]
        # Row 32g of block c holds e for rows c*CHUNK + 512g + m.
        eo = eb[[0, 32, 64, 96], :].reshape(4, NCHUNK, G)
        eo = eo.transpose(1, 0, 2).reshape(-1)[:RPC]
        e[i * RPC:(i + 1) * RPC] = \
            eo.view(ml_dtypes.float8_e4m3).astype(np.float32)
        waccs.append(res.results[i]["wacc"])

    # Denominators: segment sums of e (same fp8 values the device used).
    denom = np.bincount(bi64, weights=e.astype(np.float64), minlength=B)

    # Numerators: pure windows from device sums; boundary windows recomputed.
    # Device windows are uneven per chunk: [1536, 512] rows.
    num = np.zeros((B, D), dtype=np.float64)
    for i in range(NCORES):
        wacc_i = waccs[i]
        base = i * RPC
        for w in range(NWIN):
            cw, pw = divmod(w, 2)
            wlo = cw * CHUNK + (0 if pw == 0 else CHUNK - G)
            whi = cw * CHUNK + (CHUNK - G if pw == 0 else CHUNK)
            glo = base + wlo
            if glo >= base + RPC:
                break
            ghi = min(base + whi, base + RPC)
            b_first = bi64[glo]
            b_last = bi64[ghi - 1]
            if b_first == b_last:
                num[b_first] += wacc_i[:, w]
            else:
                sub = bi64[glo:ghi]
                cuts = np.flatnonzero(np.diff(sub)) + 1
                bounds = np.concatenate(([0], cuts, [ghi - glo]))
                for k in range(len(bounds) - 1):
                    lo, hi = glo + bounds[k], glo + bounds[k + 1]
                    num[sub[bounds[k]]] += \
                        e[lo:hi].astype(np.float64) @ x[lo:hi].astype(np.float64)

    dn = denom[:, None]
    out = np.divide(num, dn, out=np.zeros_like(num), where=dn > 0)
    out = out.astype(np.float32)
    if _want_results:
        return out, res
    return out


# revision 26
# speedup vs baseline: 1.1278x; 1.1278x over previous
"""Trainium2 Bass kernel for nn_AttentionLayer (segment softmax attention pooling).

Computation (reference):
    h = tanh(x @ W1 + b1)            # [N, A]
    s = h @ W2 + b2                  # [N, 1]
    per-segment softmax over s, out[b] = sum_i softmax_w_i * x_i   # [B, D]

Strategy (v2, bf16 data path + grouped exp):
  - Shard the N=500k instances across 8 NeuronCores (data parallel), weights
    replicated. Host pre-converts x to bf16 and pre-transposes so each core
    streams xT [D=128, rows] tiles (bf16 halves HBM traffic and doubles PE
    throughput vs the f32r baseline).
  - Per core, one pass over x, 2048-row chunks, software-pipelined:
      PE : hT = W1^T @ xT                     (4x N=512 matmuls, bf16)
      ACT: th = tanh(hT + b1) -> bf16
      PE : 4x col-tiled score matmuls -> sg [128, 512] grouped scores
           (col group g holds scores for rows 512g..512g+511 on partitions
           32g..32g+31; ACT cost is free-dim-driven, so exp on the grouped
           layout is 4x cheaper than on the broadcast layout)
      ACT: eg = exp(sg + b2) -> bf16 [128, 512]
      PE : 4x K=1 ones-matmuls broadcast eg group rows -> ebc [128, 2048]
           (PSUM) = e_i on every partition
      DVE: affine_mul_reduce(xT * ebc) summed per 1024-row window -> wacc
  - Device outputs: per-window weighted sums wacc [D, NWIN] and the e row
    (bf16, via the grouped eg tile).  Segment logic on the host: pure windows
    used directly; windows containing a segment boundary recomputed from x
    and the exported e; denominators via bincount over e.  exp without max
    subtraction is safe (scores O(+-5)); numerator/denominator share e.
"""

import numpy as np

# Problem constants (hardcoded per contract; kernel.py must be self-contained).
N = 500_000
D = 128
A = 128
B = 256
NCORES = 8
RPC = N // NCORES            # rows per core = 62500
CHUNK = 2048                 # rows per streamed tile
WIN = 1024                   # rows per reduction window
G = CHUNK // 4               # grouped score tile free dim = 512
NCHUNK = -(-RPC // CHUNK)    # 31
RPAD = NCHUNK * CHUNK        # 63488
NWIN = RPAD // WIN           # 62
MM_N = 512                   # PE moving-operand max free dim

_prog_cache = {}


def _build_program():
    import concourse.bacc as bacc
    from concourse import mybir
    from concourse.tile import TileContext

    f32 = mybir.dt.float32
    bf16 = mybir.dt.bfloat16
    fp8 = mybir.dt.float8e4
    u8 = mybir.dt.uint8
    DR = mybir.MatmulPerfMode.DoubleRow
    nc = bacc.Bacc("TRN2", target_bir_lowering=False, debug=False,
                   num_devices=NCORES)

    xt = nc.dram_tensor("xt", [D, RPAD], bf16, kind="ExternalInput")
    w1 = nc.dram_tensor("w1", [D, A], bf16, kind="ExternalInput")
    w2r = nc.dram_tensor("w2r", [A, 128], bf16, kind="ExternalInput")
    # DoubleRow lhsT pairs: [:, 0:128] = 1 (k=0 slot), [:, 128:256] = 0.
    ones2 = nc.dram_tensor("ones2", [128, 256], fp8, kind="ExternalInput")
    b1 = nc.dram_tensor("b1", [A, 1], f32, kind="ExternalInput")
    b2 = nc.dram_tensor("b2", [128, 1], f32, kind="ExternalInput")
    wacc = nc.dram_tensor("wacc", [D, NWIN], f32, kind="ExternalOutput")
    eout = nc.dram_tensor("eout", [128, NCHUNK * G], u8, kind="ExternalOutput")

    with TileContext(nc) as tc:
        with tc.tile_pool(name="const", bufs=1) as cpool, \
             tc.tile_pool(name="xtp", bufs=6) as xpool, \
             tc.tile_pool(name="thp", bufs=2) as thpool, \
             tc.tile_pool(name="junkp", bufs=2) as jpool, \
             tc.tile_pool(name="accp", bufs=1) as apool, \
             tc.tile_pool(name="psb", bufs=1, space="PSUM") as psb:

            w1sb = cpool.tile([D, A], bf16, tag="w1")
            w2rsb = cpool.tile([A, 128], bf16, tag="w2r")
            onesb = cpool.tile([128, 256], fp8, tag="ones2")
            b1sb = cpool.tile([A, 1], f32, tag="b1")
            b2sb = cpool.tile([128, 1], f32, tag="b2")
            warm = cpool.tile([128, 8], bf16, tag="warm")

            # Spread issue work across the DMA-capable engines.
            nc.vector.memset(warm[:], 0.0)
            nc.scalar.dma_start(out=w1sb[:], in_=w1[:])
            nc.sync.dma_start(out=b1sb[:], in_=b1[:])
            nc.sync.dma_start(out=b2sb[:], in_=b2[:])
            nc.sync.dma_start(out=w2rsb[:], in_=w2r[:])
            # Load the tanh/exp ACT table during the initial DMA wait.
            nc.scalar.activation(out=warm[:, 0:1], in_=warm[:, 2:3],
                                 func=mybir.ActivationFunctionType.Tanh)
            nc.scalar.activation(out=warm[:, 1:2], in_=warm[:, 2:3],
                                 func=mybir.ActivationFunctionType.Exp)
            nc.scalar.dma_start(out=onesb[:], in_=ones2[:])

            waccsb = apool.tile([D, NWIN], f32, tag="wacc")
            # Persistent store for all chunks' grouped e tiles (fp8 bytes,
            # 16 KB per partition), one junk tail block for the last chunk's
            # DoubleRow k=1 read; exported incrementally per chunk.
            eall = apool.tile([128, (NCHUNK + 1) * G], u8, tag="eall")
            nc.vector.memset(eall[:].bitcast(f32), 0.0)

            # One PSUM tensor spanning all 8 banks.  Layout per chunk:
            #   hregs = [0:1536], [1536:2048]  banks 0-2 / 3 (pre-tanh h;
            #           uneven 3:1 split so only one 512-row score group
            #           trails the second tanh -> shorter exp wait)
            #   sgreg = [CHUNK:CHUNK+G]  bank 4, overwritten by ebc g0 after
            #           exp has consumed it (true-dep aligned)
            #   ebc   = [CHUNK:2*CHUNK]  banks 4-7 (broadcast e, fp32)
            pbig = psb.tile([128, 2 * CHUNK], f32, tag="pbig")
            HCUT = 3 * MM_N
            hregs = [pbig[:, 0:HCUT], pbig[:, HCUT:CHUNK]]
            sgreg = pbig[:, CHUNK:CHUNK + G]
            ebc = pbig[:, CHUNK:2 * CHUNK]

            # Warm the PE's HAM clock gate during the initial DMA wait:
            # junk matmuls (inputs: the zeroed eall slice; output: the sgreg
            # scratch, later cleared by score(0)'s start=True).
            for _ in range(5):
                nc.tensor.matmul(out=sgreg[:],
                                 lhsT=eall[:, 0:256].bitcast(bf16),
                                 rhs=eall[:, 0:2 * MM_N].bitcast(bf16),
                                 start=True, stop=True)

            xtiles, ths = {}, {}

            def load_chunk(c, split=False):
                xtile = xpool.tile([D, CHUNK], bf16, tag="x")
                base = c * CHUNK
                if split:
                    # First chunk: quarter-loads issued from different
                    # engines so the first h-matmul unblocks after 128 KB
                    # and the issues don't serialize.
                    engines = [nc.sync, nc.scalar, nc.gpsimd, nc.sync]
                    for q, eng in enumerate(engines):
                        eng.dma_start(
                            out=xtile[:, q * MM_N:(q + 1) * MM_N],
                            in_=xt[:, base + q * MM_N:base + (q + 1) * MM_N])
                else:
                    nc.gpsimd.dma_start(out=xtile[:],
                                        in_=xt[:, base:base + CHUNK])
                xtiles[c] = xtile

            def stage_scores(p):
                # 4 col-tiled score matmuls: group g writes scores for rows
                # 512g..512g+511 onto partitions 32g..32g+31 (bank 4).
                th_p = ths[p]
                for g in range(4):
                    nc.tensor.matmul(
                        out=sgreg[32 * g:32 * (g + 1), :],
                        lhsT=w2rsb[:, 32 * g:32 * (g + 1)],
                        rhs=th_p[:, G * g:G * (g + 1)],
                        start=True, stop=True,
                        tile_position=(0, 32 * g))
                eg = eall[:, p * G:(p + 1) * G].bitcast(fp8)
                nc.scalar.activation(out=eg, in_=sgreg,
                                     func=mybir.ActivationFunctionType.Exp,
                                     bias=b2sb[:, 0:1])

            def stage_h(c):
                # h matmuls + tanh in uneven (1536, 512) parts so the PE
                # overlaps tanh, and only score group 3 trails the second
                # (short) tanh.
                xtile = xtiles[c]
                th = thpool.tile([A, CHUNK], bf16, tag="th")
                ths[c] = th
                for hh, (lo, hi) in enumerate(((0, HCUT), (HCUT, CHUNK))):
                    for off in range(lo, hi, MM_N):
                        nc.tensor.matmul(
                            out=hregs[hh][:, off - lo:off - lo + MM_N],
                            lhsT=w1sb[:],
                            rhs=xtile[:, off:off + MM_N],
                            start=True, stop=True)
                    nc.scalar.activation(
                        out=th[:, lo:hi],
                        in_=hregs[hh],
                        func=mybir.ActivationFunctionType.Tanh,
                        bias=b1sb[:, 0:1])

            def stage_tail(p):
                # Broadcast e via fp8 DoubleRow ones-matmuls (half the bf16
                # streaming cost; the k=1 lhsT slot is zero so the adjacent
                # e block read as the k=1 operand is inert), then the
                # windowed weighted reductions.
                xtile_p = xtiles.pop(p)
                ths.pop(p)
                for g in range(4):
                    rhs = eall[32 * g:32 * g + 1,
                               p * G:(p + 2) * G].bitcast(fp8).rearrange(
                                   "q (k n) -> q k n", k=2)
                    lhsT = onesb[32 * g:32 * g + 1, :].rearrange(
                        "q (k m) -> q k m", k=2)
                    nc.tensor.matmul(
                        out=ebc[:, G * g:G * (g + 1)],
                        lhsT=lhsT, rhs=rhs,
                        start=True, stop=True,
                        perf_mode=DR,
                        tile_position=(32 * g, 0))
                for w in range(CHUNK // WIN):
                    gw = p * (CHUNK // WIN) + w
                    junk = jpool.tile([D, 1], f32, tag="junk")
                    nc.vector.affine_mul_reduce(
                        out=junk[:].to_broadcast([D, WIN]),
                        accum_out=waccsb[:, gw:gw + 1],
                        in0=xtile_p[:, w * WIN:(w + 1) * WIN],
                        in1=ebc[:, w * WIN:(w + 1) * WIN],
                        scale=1.0,
                        bias=0.0)
                # Incremental export: this chunk's e block and wacc columns
                # leave now so the kernel tail is just the pipeline drain.
                nc.sync.dma_start(out=eout[:, p * G:(p + 1) * G],
                                  in_=eall[:, p * G:(p + 1) * G])
                nc.sync.dma_start(
                    out=wacc[:, 2 * p:2 * p + 2],
                    in_=waccsb[:, 2 * p:2 * p + 2])

            load_chunk(0, split=True)
            for cc in (1, 2):
                if cc < NCHUNK:
                    load_chunk(cc)
            # Period structure: [ebc(c-1) + windows(c-1)] | [h(c) + tanh(c)]
            # | [score(c) + exp(c)].  The e-broadcast matmuls run first each
            # period (their inputs finished last period), the windows drain
            # on DVE mid-period, and score/exp land at the end — so no
            # cross-engine cycle spans more than one period.
            stage_h(0)
            stage_scores(0)
            for c in range(1, NCHUNK + 1):
                if c + 2 < NCHUNK:
                    load_chunk(c + 2)
                stage_tail(c - 1)
                if c < NCHUNK:
                    stage_h(c)
                    stage_scores(c)

    nc.compile()
    return nc


def _to_bf16(a):
    import ml_dtypes
    return np.asarray(a).astype(ml_dtypes.bfloat16)


def _run_device(xt_shards, W1, W2, b1, b2, trace=False):
    from concourse.bass_utils import run_bass_kernel_spmd
    import ml_dtypes

    if "prog" not in _prog_cache:
        _prog_cache["prog"] = _build_program()
    nc = _prog_cache["prog"]

    w1_in = np.ascontiguousarray(_to_bf16(W1))
    w2r_in = np.ascontiguousarray(
        _to_bf16(np.tile(np.asarray(W2, dtype=np.float32).reshape(A, 1),
                         (1, 128))))
    ones2_in = np.zeros((128, 256), dtype=ml_dtypes.float8_e4m3)
    ones2_in[:, 0:128] = 1.0
    b1_in = np.ascontiguousarray(np.asarray(b1, np.float32).reshape(A, 1))
    b2_in = np.full((128, 1), np.float32(b2), dtype=np.float32)

    in_maps = [{"xt": xt_shards[i], "w1": w1_in, "w2r": w2r_in,
                "ones2": ones2_in, "b1": b1_in, "b2": b2_in}
               for i in range(NCORES)]
    res = run_bass_kernel_spmd(nc, in_maps, core_ids=list(range(NCORES)),
                               trace=trace)
    return res


def kernel(x, batch_index, W1, b1, W2, b2, _want_results=False, _trace=False):
    import ml_dtypes

    x = np.asarray(x, dtype=np.float32)
    bi64 = np.asarray(batch_index).astype(np.int64)
    b2v = float(np.asarray(b2, dtype=np.float32).reshape(-1)[0])

    assert x.shape == (N, D)

    # Host pre-transpose + bf16: xT [D, N] bf16, then per-core padded shards.
    xb = x.astype(ml_dtypes.bfloat16)
    xtb = xb.T
    xt_shards = []
    for i in range(NCORES):
        sh = np.zeros((D, RPAD), dtype=ml_dtypes.bfloat16)
        sh[:, :RPC] = xtb[:, i * RPC:(i + 1) * RPC]
        xt_shards.append(np.ascontiguousarray(sh))

    res = _run_device(xt_shards, W1, W2, b1, b2v, trace=_trace)

    # Gather device outputs.
    e = np.empty(N, dtype=np.float32)
    waccs = []
    for i in range(NCORES):
        eb = res.results[i]["eout"]  # uint8 fp8e4m3 bytes [128, NCHUNK*G]
        # Row 32g of block p holds e for rows p*CHUNK + 512g + m: regroup
        # to chunk-major order and decode the fp8 bytes.
        eo = eb[[0, 32, 64, 96], :].reshape(4, NCHUNK, G)
        eo = eo.transpose(1, 0, 2).reshape(-1)[:RPC]
        e[i * RPC:(i + 1) * RPC] = \
            eo.view(ml_dtypes.float8_e4m3).astype(np.float32)
        waccs.append(res.results[i]["wacc"])

    # Denominators: segment sums of e (same bf16 values the device used).
    denom = np.bincount(bi64, weights=e.astype(np.float64), minlength=B)

    # Numerators: pure windows from device sums; boundary windows recomputed.
    num = np.zeros((B, D), dtype=np.float64)
    for i in range(NCORES):
        wacc_i = waccs[i]
        base = i * RPC
        for w in range(NWIN):
            glo = base + w * WIN
            if glo >= base + RPC:
                break
            ghi = min(glo + WIN, base + RPC)
            b_first = bi64[glo]
            b_last = bi64[ghi - 1]
            if b_first == b_last:
                num[b_first] += wacc_i[:, w]
            else:
                sub = bi64[glo:ghi]
                cuts = np.flatnonzero(np.diff(sub)) + 1
                bounds = np.concatenate(([0], cuts, [ghi - glo]))
                for k in range(len(bounds) - 1):
                    lo, hi = glo + bounds[k], glo + bounds[k + 1]
                    num[sub[bounds[k]]] += \
                        e[lo:hi].astype(np.float64) @ x[lo:hi].astype(np.float64)

    dn = denom[:, None]
    out = np.divide(num, dn, out=np.zeros_like(num), where=dn > 0)
    out = out.astype(np.float32)
    if _want_results:
        return out, res
    return out



# revision 27
# speedup vs baseline: 1.2403x; 1.0998x over previous
"""Trainium2 Bass kernel for nn_AttentionLayer (segment softmax attention pooling).

Computation (reference):
    h = tanh(x @ W1 + b1)            # [N, A]
    s = h @ W2 + b2                  # [N, 1]
    per-segment softmax over s, out[b] = sum_i softmax_w_i * x_i   # [B, D]

Strategy (v2, bf16 data path + grouped exp):
  - Shard the N=500k instances across 8 NeuronCores (data parallel), weights
    replicated. Host pre-converts x to bf16 and pre-transposes so each core
    streams xT [D=128, rows] tiles (bf16 halves HBM traffic and doubles PE
    throughput vs the f32r baseline).
  - Per core, one pass over x, 2048-row chunks, software-pipelined:
      PE : hT = W1^T @ xT                     (4x N=512 matmuls, bf16)
      ACT: th = tanh(hT + b1) -> bf16
      PE : 4x col-tiled score matmuls -> sg [128, 512] grouped scores
           (col group g holds scores for rows 512g..512g+511 on partitions
           32g..32g+31; ACT cost is free-dim-driven, so exp on the grouped
           layout is 4x cheaper than on the broadcast layout)
      ACT: eg = exp(sg + b2) -> bf16 [128, 512]
      PE : 4x K=1 ones-matmuls broadcast eg group rows -> ebc [128, 2048]
           (PSUM) = e_i on every partition
      DVE: affine_mul_reduce(xT * ebc) summed per 1024-row window -> wacc
  - Device outputs: per-window weighted sums wacc [D, NWIN] and the e row
    (bf16, via the grouped eg tile).  Segment logic on the host: pure windows
    used directly; windows containing a segment boundary recomputed from x
    and the exported e; denominators via bincount over e.  exp without max
    subtraction is safe (scores O(+-5)); numerator/denominator share e.
"""

import numpy as np

# Problem constants (hardcoded per contract; kernel.py must be self-contained).
N = 500_000
D = 128
A = 128
B = 256
NCORES = 8
RPC = N // NCORES            # rows per core = 62500
CHUNK = 2048                 # rows per streamed tile
WIN = 1024                   # rows per reduction window
G = CHUNK // 4               # grouped score tile free dim = 512
NCHUNK = -(-RPC // CHUNK)    # 31
RPAD = NCHUNK * CHUNK        # 63488
NWIN = RPAD // WIN           # 62
MM_N = 512                   # PE moving-operand max free dim

_prog_cache = {}


def _build_program():
    import concourse.bacc as bacc
    from concourse import mybir
    from concourse.tile import TileContext

    f32 = mybir.dt.float32
    bf16 = mybir.dt.bfloat16
    fp8 = mybir.dt.float8e4
    u8 = mybir.dt.uint8
    DR = mybir.MatmulPerfMode.DoubleRow
    nc = bacc.Bacc("TRN2", target_bir_lowering=False, debug=False,
                   num_devices=NCORES)

    xt = nc.dram_tensor("xt", [D, RPAD], bf16, kind="ExternalInput")
    w1 = nc.dram_tensor("w1", [D, A], bf16, kind="ExternalInput")
    w2r = nc.dram_tensor("w2r", [A, 128], bf16, kind="ExternalInput")
    # DoubleRow lhsT pairs: [:, 0:128] = 1 (k=0 slot), [:, 128:256] = 0.
    ones2 = nc.dram_tensor("ones2", [128, 256], fp8, kind="ExternalInput")
    b1 = nc.dram_tensor("b1", [A, 1], f32, kind="ExternalInput")
    b2 = nc.dram_tensor("b2", [128, 1], f32, kind="ExternalInput")
    wacc = nc.dram_tensor("wacc", [D, NWIN], f32, kind="ExternalOutput")
    eout = nc.dram_tensor("eout", [128, NCHUNK * G], u8, kind="ExternalOutput")

    with TileContext(nc) as tc:
        with tc.tile_pool(name="const", bufs=1) as cpool, \
             tc.tile_pool(name="xtp", bufs=6) as xpool, \
             tc.tile_pool(name="thp", bufs=2) as thpool, \
             tc.tile_pool(name="junkp", bufs=2) as jpool, \
             tc.tile_pool(name="accp", bufs=1) as apool, \
             tc.tile_pool(name="psb", bufs=1, space="PSUM") as psb:

            w1sb = cpool.tile([D, A], bf16, tag="w1")
            w2rsb = cpool.tile([A, 128], bf16, tag="w2r")
            onesb = cpool.tile([128, 256], fp8, tag="ones2")
            b1sb = cpool.tile([A, 1], f32, tag="b1")
            b2sb = cpool.tile([128, 1], f32, tag="b2")
            warm = cpool.tile([128, 8], bf16, tag="warm")

            # Spread issue work across the DMA-capable engines.
            nc.vector.memset(warm[:], 0.0)
            nc.scalar.dma_start(out=w1sb[:], in_=w1[:])
            nc.sync.dma_start(out=b1sb[:], in_=b1[:])
            nc.sync.dma_start(out=b2sb[:], in_=b2[:])
            nc.sync.dma_start(out=w2rsb[:], in_=w2r[:])
            # Load the tanh/exp ACT table during the initial DMA wait.
            nc.scalar.activation(out=warm[:, 0:1], in_=warm[:, 2:3],
                                 func=mybir.ActivationFunctionType.Tanh)
            nc.scalar.activation(out=warm[:, 1:2], in_=warm[:, 2:3],
                                 func=mybir.ActivationFunctionType.Exp)
            nc.scalar.dma_start(out=onesb[:], in_=ones2[:])

            waccsb = apool.tile([D, NWIN], f32, tag="wacc")
            # Persistent store for all chunks' grouped e tiles (fp8 bytes,
            # 16 KB per partition), one junk tail block for the last chunk's
            # DoubleRow k=1 read; exported incrementally per chunk.
            eall = apool.tile([128, (NCHUNK + 1) * G], u8, tag="eall")
            nc.vector.memset(eall[:].bitcast(f32), 0.0)

            # One PSUM tensor spanning all 8 banks.  Layout per chunk:
            #   hregs = [0:1536], [1536:2048]  banks 0-2 / 3 (pre-tanh h;
            #           uneven 3:1 split so only one 512-row score group
            #           trails the second tanh -> shorter exp wait)
            #   sgreg = [CHUNK:CHUNK+G]  bank 4, overwritten by ebc g0 after
            #           exp has consumed it (true-dep aligned)
            #   ebc   = [CHUNK:2*CHUNK]  banks 4-7 (broadcast e, fp32)
            pbig = psb.tile([128, 2 * CHUNK], f32, tag="pbig")
            HCUT = CHUNK // 2
            hregs = [pbig[:, 0:HCUT], pbig[:, HCUT:CHUNK]]
            sgreg = pbig[:, CHUNK:CHUNK + G]
            ebc = pbig[:, CHUNK:2 * CHUNK]

            # Warm the PE's HAM clock gate during the initial DMA wait:
            # junk matmuls (inputs: the zeroed eall slice; output: the sgreg
            # scratch, later cleared by score(0)'s start=True).
            for _ in range(5):
                nc.tensor.matmul(out=sgreg[:],
                                 lhsT=eall[:, 0:256].bitcast(bf16),
                                 rhs=eall[:, 0:2 * MM_N].bitcast(bf16),
                                 start=True, stop=True)

            xtiles, ths = {}, {}

            def load_chunk(c, split=False):
                xtile = xpool.tile([D, CHUNK], bf16, tag="x")
                base = c * CHUNK
                if split:
                    # First chunk: quarter-loads issued from different
                    # engines so the first h-matmul unblocks after 128 KB
                    # and the issues don't serialize.
                    engines = [nc.sync, nc.scalar, nc.gpsimd, nc.sync]
                    for q, eng in enumerate(engines):
                        eng.dma_start(
                            out=xtile[:, q * MM_N:(q + 1) * MM_N],
                            in_=xt[:, base + q * MM_N:base + (q + 1) * MM_N])
                else:
                    nc.gpsimd.dma_start(out=xtile[:],
                                        in_=xt[:, base:base + CHUNK])
                xtiles[c] = xtile

            def stage_scores(p):
                # 4 col-tiled score matmuls: group g writes scores for rows
                # 512g..512g+511 onto partitions 32g..32g+31 (bank 4).
                th_p = ths[p]
                for g in range(4):
                    nc.tensor.matmul(
                        out=sgreg[32 * g:32 * (g + 1), :],
                        lhsT=w2rsb[:, 32 * g:32 * (g + 1)],
                        rhs=th_p[:, G * g:G * (g + 1)],
                        start=True, stop=True,
                        tile_position=(0, 32 * g))
                eg = eall[:, p * G:(p + 1) * G].bitcast(fp8)
                nc.scalar.activation(out=eg, in_=sgreg,
                                     func=mybir.ActivationFunctionType.Exp,
                                     bias=b2sb[:, 0:1])

            def stage_h(c):
                # h matmuls + tanh in uneven (1536, 512) parts so the PE
                # overlaps tanh, and only score group 3 trails the second
                # (short) tanh.
                xtile = xtiles[c]
                th = thpool.tile([A, CHUNK], bf16, tag="th")
                ths[c] = th
                for hh, (lo, hi) in enumerate(((0, HCUT), (HCUT, CHUNK))):
                    for off in range(lo, hi, MM_N):
                        nc.tensor.matmul(
                            out=hregs[hh][:, off - lo:off - lo + MM_N],
                            lhsT=w1sb[:],
                            rhs=xtile[:, off:off + MM_N],
                            start=True, stop=True)
                    nc.scalar.activation(
                        out=th[:, lo:hi],
                        in_=hregs[hh],
                        func=mybir.ActivationFunctionType.Tanh,
                        bias=b1sb[:, 0:1])

            def stage_tail(p):
                # Broadcast e via fp8 DoubleRow ones-matmuls (half the bf16
                # streaming cost; the k=1 lhsT slot is zero so the adjacent
                # e block read as the k=1 operand is inert), then the
                # windowed weighted reductions.
                xtile_p = xtiles.pop(p)
                ths.pop(p)
                for g in range(4):
                    rhs = eall[32 * g:32 * g + 1,
                               p * G:(p + 2) * G].bitcast(fp8).rearrange(
                                   "q (k n) -> q k n", k=2)
                    lhsT = onesb[32 * g:32 * g + 1, :].rearrange(
                        "q (k m) -> q k m", k=2)
                    nc.tensor.matmul(
                        out=ebc[:, G * g:G * (g + 1)],
                        lhsT=lhsT, rhs=rhs,
                        start=True, stop=True,
                        perf_mode=DR,
                        tile_position=(32 * g, 0))
                for w in range(CHUNK // WIN):
                    gw = p * (CHUNK // WIN) + w
                    junk = jpool.tile([D, 1], f32, tag="junk")
                    nc.vector.affine_mul_reduce(
                        out=junk[:].to_broadcast([D, WIN]),
                        accum_out=waccsb[:, gw:gw + 1],
                        in0=xtile_p[:, w * WIN:(w + 1) * WIN],
                        in1=ebc[:, w * WIN:(w + 1) * WIN],
                        scale=1.0,
                        bias=0.0)
                # Incremental export: this chunk's e block and wacc columns
                # leave now so the kernel tail is just the pipeline drain.
                nc.sync.dma_start(out=eout[:, p * G:(p + 1) * G],
                                  in_=eall[:, p * G:(p + 1) * G])
                nc.sync.dma_start(
                    out=wacc[:, 2 * p:2 * p + 2],
                    in_=waccsb[:, 2 * p:2 * p + 2])

            load_chunk(0, split=True)
            for cc in (1, 2):
                if cc < NCHUNK:
                    load_chunk(cc)
            # Period structure: [ebc(c-1) + windows(c-1)] | [h(c) + tanh(c)]
            # | [score(c) + exp(c)].  The e-broadcast matmuls run first each
            # period (their inputs finished last period), the windows drain
            # on DVE mid-period, and score/exp land at the end — so no
            # cross-engine cycle spans more than one period.
            stage_h(0)
            stage_scores(0)
            for c in range(1, NCHUNK + 1):
                if c + 2 < NCHUNK:
                    load_chunk(c + 2)
                stage_tail(c - 1)
                if c < NCHUNK:
                    stage_h(c)
                    stage_scores(c)

    nc.compile()
    return nc


def _to_bf16(a):
    import ml_dtypes
    return np.asarray(a).astype(ml_dtypes.bfloat16)


def _run_device(xt_shards, W1, W2, b1, b2, trace=False):
    from concourse.bass_utils import run_bass_kernel_spmd
    import ml_dtypes

    if "prog" not in _prog_cache:
        _prog_cache["prog"] = _build_program()
    nc = _prog_cache["prog"]

    w1_in = np.ascontiguousarray(_to_bf16(W1))
    w2r_in = np.ascontiguousarray(
        _to_bf16(np.tile(np.asarray(W2, dtype=np.float32).reshape(A, 1),
                         (1, 128))))
    ones2_in = np.zeros((128, 256), dtype=ml_dtypes.float8_e4m3)
    ones2_in[:, 0:128] = 1.0
    b1_in = np.ascontiguousarray(np.asarray(b1, np.float32).reshape(A, 1))
    b2_in = np.full((128, 1), np.float32(b2), dtype=np.float32)

    in_maps = [{"xt": xt_shards[i], "w1": w1_in, "w2r": w2r_in,
                "ones2": ones2_in, "b1": b1_in, "b2": b2_in}
               for i in range(NCORES)]
    res = run_bass_kernel_spmd(nc, in_maps, core_ids=list(range(NCORES)),
                               trace=trace)
    return res


def kernel(x, batch_index, W1, b1, W2, b2, _want_results=False, _trace=False):
    import ml_dtypes

    x = np.asarray(x, dtype=np.float32)
    bi64 = np.asarray(batch_index).astype(np.int64)
    b2v = float(np.asarray(b2, dtype=np.float32).reshape(-1)[0])

    assert x.shape == (N, D)

    # Host pre-transpose + bf16: xT [D, N] bf16, then per-core padded shards.
    xb = x.astype(ml_dtypes.bfloat16)
    xtb = xb.T
    xt_shards = []
    for i in range(NCORES):
        sh = np.zeros((D, RPAD), dtype=ml_dtypes.bfloat16)
        sh[:, :RPC] = xtb[:, i * RPC:(i + 1) * RPC]
        xt_shards.append(np.ascontiguousarray(sh))

    res = _run_device(xt_shards, W1, W2, b1, b2v, trace=_trace)

    # Gather device outputs.
    e = np.empty(N, dtype=np.float32)
    waccs = []
    for i in range(NCORES):
        eb = res.results[i]["eout"]  # uint8 fp8e4m3 bytes [128, NCHUNK*G]
        # Row 32g of block p holds e for rows p*CHUNK + 512g + m: regroup
        # to chunk-major order and decode the fp8 bytes.
        eo = eb[[0, 32, 64, 96], :].reshape(4, NCHUNK, G)
        eo = eo.transpose(1, 0, 2).reshape(-1)[:RPC]
        e[i * RPC:(i + 1) * RPC] = \
            eo.view(ml_dtypes.float8_e4m3).astype(np.float32)
        waccs.append(res.results[i]["wacc"])

    # Denominators: segment sums of e (same bf16 values the device used).
    denom = np.bincount(bi64, weights=e.astype(np.float64), minlength=B)

    # Numerators: pure windows from device sums; boundary windows recomputed.
    num = np.zeros((B, D), dtype=np.float64)
    for i in range(NCORES):
        wacc_i = waccs[i]
        base = i * RPC
        for w in range(NWIN):
            glo = base + w * WIN
            if glo >= base + RPC:
                break
            ghi = min(glo + WIN, base + RPC)
            b_first = bi64[glo]
            b_last = bi64[ghi - 1]
            if b_first == b_last:
                num[b_first] += wacc_i[:, w]
            else:
                sub = bi64[glo:ghi]
                cuts = np.flatnonzero(np.diff(sub)) + 1
                bounds = np.concatenate(([0], cuts, [ghi - glo]))
                for k in range(len(bounds) - 1):
                    lo, hi = glo + bounds[k], glo + bounds[k + 1]
                    num[sub[bounds[k]]] += \
                        e[lo:hi].astype(np.float64) @ x[lo:hi].astype(np.float64)

    dn = denom[:, None]
    out = np.divide(num, dn, out=np.zeros_like(num), where=dn > 0)
    out = out.astype(np.float32)
    if _want_results:
        return out, res
    return out



# revision 28
# speedup vs baseline: 1.2811x; 1.0329x over previous
"""Trainium2 Bass kernel for nn_AttentionLayer (segment softmax attention pooling).

Computation (reference):
    h = tanh(x @ W1 + b1)            # [N, A]
    s = h @ W2 + b2                  # [N, 1]
    per-segment softmax over s, out[b] = sum_i softmax_w_i * x_i   # [B, D]

Strategy (v2, bf16 data path + grouped exp):
  - Shard the N=500k instances across 8 NeuronCores (data parallel), weights
    replicated. Host pre-converts x to bf16 and pre-transposes so each core
    streams xT [D=128, rows] tiles (bf16 halves HBM traffic and doubles PE
    throughput vs the f32r baseline).
  - Per core, one pass over x, 2048-row chunks, software-pipelined:
      PE : hT = W1^T @ xT                     (4x N=512 matmuls, bf16)
      ACT: th = tanh(hT + b1) -> bf16
      PE : 4x col-tiled score matmuls -> sg [128, 512] grouped scores
           (col group g holds scores for rows 512g..512g+511 on partitions
           32g..32g+31; ACT cost is free-dim-driven, so exp on the grouped
           layout is 4x cheaper than on the broadcast layout)
      ACT: eg = exp(sg + b2) -> bf16 [128, 512]
      PE : 4x K=1 ones-matmuls broadcast eg group rows -> ebc [128, 2048]
           (PSUM) = e_i on every partition
      DVE: affine_mul_reduce(xT * ebc) summed per 1024-row window -> wacc
  - Device outputs: per-window weighted sums wacc [D, NWIN] and the e row
    (bf16, via the grouped eg tile).  Segment logic on the host: pure windows
    used directly; windows containing a segment boundary recomputed from x
    and the exported e; denominators via bincount over e.  exp without max
    subtraction is safe (scores O(+-5)); numerator/denominator share e.
"""

import numpy as np

# Problem constants (hardcoded per contract; kernel.py must be self-contained).
N = 500_000
D = 128
A = 128
B = 256
NCORES = 8
RPC = N // NCORES            # rows per core = 62500
CHUNK = 2048                 # rows per streamed tile
WIN = 1024                   # rows per reduction window
G = CHUNK // 4               # grouped score tile free dim = 512
NCHUNK = -(-RPC // CHUNK)    # 31
RPAD = NCHUNK * CHUNK        # 63488
NWIN = RPAD // WIN           # 62
MM_N = 512                   # PE moving-operand max free dim

_prog_cache = {}


def _build_program():
    import concourse.bacc as bacc
    from concourse import mybir
    from concourse.tile import TileContext

    f32 = mybir.dt.float32
    bf16 = mybir.dt.bfloat16
    nc = bacc.Bacc("TRN2", target_bir_lowering=False, debug=False,
                   num_devices=NCORES)

    xt = nc.dram_tensor("xt", [D, RPAD], bf16, kind="ExternalInput")
    w1 = nc.dram_tensor("w1", [D, A], bf16, kind="ExternalInput")
    w2r = nc.dram_tensor("w2r", [A, 128], bf16, kind="ExternalInput")
    ones1 = nc.dram_tensor("ones1", [128, 128], bf16, kind="ExternalInput")
    b1 = nc.dram_tensor("b1", [A, 1], f32, kind="ExternalInput")
    b2 = nc.dram_tensor("b2", [128, 1], f32, kind="ExternalInput")
    wacc = nc.dram_tensor("wacc", [D, NWIN], f32, kind="ExternalOutput")
    eout = nc.dram_tensor("eout", [4, NCHUNK * G], bf16, kind="ExternalOutput")

    with TileContext(nc) as tc:
        with tc.tile_pool(name="const", bufs=1) as cpool, \
             tc.tile_pool(name="xtp", bufs=6) as xpool, \
             tc.tile_pool(name="thp", bufs=2) as thpool, \
             tc.tile_pool(name="junkp", bufs=2) as jpool, \
             tc.tile_pool(name="accp", bufs=1) as apool, \
             tc.tile_pool(name="psb", bufs=1, space="PSUM") as psb:

            w1sb = cpool.tile([D, A], bf16, tag="w1")
            w2rsb = cpool.tile([A, 128], bf16, tag="w2r")
            onesb = cpool.tile([128, 128], bf16, tag="ones1")
            b1sb = cpool.tile([A, 1], f32, tag="b1")
            b2sb = cpool.tile([128, 1], f32, tag="b2")

            nc.sync.dma_start(out=w1sb[:], in_=w1[:])
            nc.sync.dma_start(out=b1sb[:], in_=b1[:])
            nc.sync.dma_start(out=w2rsb[:], in_=w2r[:])
            nc.sync.dma_start(out=b2sb[:], in_=b2[:])
            nc.sync.dma_start(out=onesb[:], in_=ones1[:])

            waccsb = apool.tile([D, NWIN], f32, tag="wacc")
            nc.vector.memset(waccsb[:], 0.0)
            # Persistent store for all chunks' grouped e tiles (bf16, 31 KB
            # per partition); exported once at the end.
            eall = apool.tile([128, NCHUNK * G], bf16, tag="eall")
            nc.vector.memset(eall[:, 0:MM_N].bitcast(f32), 1.0)

            # One PSUM tensor spanning all 8 banks.  Layout per chunk:
            #   hregs = [0:1024], [1024:2048]   banks 0-1 / 2-3 (pre-tanh h,
            #           two half-chunk buffers so PE overlaps tanh)
            #   sgreg = [CHUNK:CHUNK+G]  bank 4, overwritten by ebc g0 after
            #           exp has consumed it (true-dep aligned)
            #   ebc   = [CHUNK:2*CHUNK]  banks 4-7 (broadcast e, fp32)
            pbig = psb.tile([128, 2 * CHUNK], f32, tag="pbig")
            HALF = CHUNK // 2
            hregs = [pbig[:, 0:HALF], pbig[:, HALF:CHUNK]]
            sgreg = pbig[:, CHUNK:CHUNK + G]
            ebc = pbig[:, CHUNK:2 * CHUNK]

            # Warm the PE's HAM clock gate during the initial DMA wait:
            # ~5 us of junk matmuls (inputs: the memset slice of eall; output:
            # the sgreg scratch, later cleared by score(0)'s start=True).
            for _ in range(8):
                nc.tensor.matmul(out=sgreg[:],
                                 lhsT=eall[:, 0:128],
                                 rhs=eall[:, 0:MM_N],
                                 start=True, stop=True)

            xtiles, ths = {}, {}

            def load_chunk(c, split=False):
                xtile = xpool.tile([D, CHUNK], bf16, tag="x")
                base = c * CHUNK
                if split:
                    # First chunk: quarter-loads so the first h-matmul
                    # unblocks after 128 KB instead of 512 KB.
                    for q in range(4):
                        nc.gpsimd.dma_start(
                            out=xtile[:, q * MM_N:(q + 1) * MM_N],
                            in_=xt[:, base + q * MM_N:base + (q + 1) * MM_N])
                else:
                    nc.gpsimd.dma_start(out=xtile[:],
                                        in_=xt[:, base:base + CHUNK])
                xtiles[c] = xtile

            def stage_scores(p):
                # 4 col-tiled score matmuls: group g writes scores for rows
                # 512g..512g+511 onto partitions 32g..32g+31 (bank 4).
                th_p = ths[p]
                for g in range(4):
                    nc.tensor.matmul(
                        out=sgreg[32 * g:32 * (g + 1), :],
                        lhsT=w2rsb[:, 32 * g:32 * (g + 1)],
                        rhs=th_p[:, G * g:G * (g + 1)],
                        start=True, stop=True,
                        tile_position=(0, 32 * g))
                eg = eall[:, p * G:(p + 1) * G]
                nc.scalar.activation(out=eg, in_=sgreg,
                                     func=mybir.ActivationFunctionType.Exp,
                                     bias=b2sb[:, 0:1])

            def stage_h(c):
                # h matmuls + tanh in half-chunks so the PE can start the
                # next half/chunk while ACT runs tanh on the previous one.
                xtile = xtiles[c]
                th = thpool.tile([A, CHUNK], bf16, tag="th")
                ths[c] = th
                for hh in range(2):
                    off = hh * HALF
                    for i in range(HALF // MM_N):
                        nc.tensor.matmul(
                            out=hregs[hh][:, i * MM_N:(i + 1) * MM_N],
                            lhsT=w1sb[:],
                            rhs=xtile[:, off + i * MM_N:off + (i + 1) * MM_N],
                            start=True, stop=True)
                    nc.scalar.activation(
                        out=th[:, off:off + HALF],
                        in_=hregs[hh],
                        func=mybir.ActivationFunctionType.Tanh,
                        bias=b1sb[:, 0:1])

            def stage_tail(p):
                # Broadcast e via K=1 row-tiled ones-matmuls, then the
                # windowed weighted reductions.
                xtile_p = xtiles.pop(p)
                ths.pop(p)
                eg = eall[:, p * G:(p + 1) * G]
                for g in range(4):
                    nc.tensor.matmul(
                        out=ebc[:, G * g:G * (g + 1)],
                        lhsT=onesb[32 * g:32 * g + 1, :],
                        rhs=eg[32 * g:32 * g + 1, :],
                        start=True, stop=True,
                        tile_position=(32 * g, 0))
                for w in range(CHUNK // WIN):
                    gw = p * (CHUNK // WIN) + w
                    junk = jpool.tile([D, 1], f32, tag="junk")
                    nc.vector.affine_mul_reduce(
                        out=junk[:].to_broadcast([D, WIN]),
                        accum_out=waccsb[:, gw:gw + 1],
                        in0=xtile_p[:, w * WIN:(w + 1) * WIN],
                        in1=ebc[:, w * WIN:(w + 1) * WIN],
                        scale=1.0,
                        bias=0.0)

            load_chunk(0, split=True)
            for cc in (1, 2):
                if cc < NCHUNK:
                    load_chunk(cc)
            # Period structure: [ebc(c-1) + windows(c-1)] | [h(c) + tanh(c)]
            # | [score(c) + exp(c)].  The e-broadcast matmuls run first each
            # period (their inputs finished last period), the windows drain
            # on DVE mid-period, and score/exp land at the end — so no
            # cross-engine cycle spans more than one period.
            stage_h(0)
            stage_scores(0)
            for c in range(1, NCHUNK + 1):
                if c + 2 < NCHUNK:
                    load_chunk(c + 2)
                stage_tail(c - 1)
                if c < NCHUNK:
                    stage_h(c)
                    stage_scores(c)

            for g in range(4):
                nc.sync.dma_start(out=eout[g:g + 1, :],
                                  in_=eall[32 * g:32 * g + 1, :])
            nc.sync.dma_start(out=wacc[:], in_=waccsb[:])

    nc.compile()
    return nc


def _to_bf16(a):
    import ml_dtypes
    return np.asarray(a).astype(ml_dtypes.bfloat16)


def _run_device(xt_shards, W1, W2, b1, b2, trace=False):
    from concourse.bass_utils import run_bass_kernel_spmd
    import ml_dtypes

    if "prog" not in _prog_cache:
        _prog_cache["prog"] = _build_program()
    nc = _prog_cache["prog"]

    w1_in = np.ascontiguousarray(_to_bf16(W1))
    w2r_in = np.ascontiguousarray(
        _to_bf16(np.tile(np.asarray(W2, dtype=np.float32).reshape(A, 1),
                         (1, 128))))
    ones_in = np.ones((128, 128), dtype=ml_dtypes.bfloat16)
    b1_in = np.ascontiguousarray(np.asarray(b1, np.float32).reshape(A, 1))
    b2_in = np.full((128, 1), np.float32(b2), dtype=np.float32)

    in_maps = [{"xt": xt_shards[i], "w1": w1_in, "w2r": w2r_in,
                "ones1": ones_in, "b1": b1_in, "b2": b2_in}
               for i in range(NCORES)]
    res = run_bass_kernel_spmd(nc, in_maps, core_ids=list(range(NCORES)),
                               trace=trace)
    return res


def kernel(x, batch_index, W1, b1, W2, b2, _want_results=False, _trace=False):
    import ml_dtypes

    x = np.asarray(x, dtype=np.float32)
    bi64 = np.asarray(batch_index).astype(np.int64)
    b2v = float(np.asarray(b2, dtype=np.float32).reshape(-1)[0])

    assert x.shape == (N, D)

    # Host pre-transpose + bf16: xT [D, N] bf16, then per-core padded shards.
    xb = x.astype(ml_dtypes.bfloat16)
    xtb = xb.T
    xt_shards = []
    for i in range(NCORES):
        sh = np.zeros((D, RPAD), dtype=ml_dtypes.bfloat16)
        sh[:, :RPC] = xtb[:, i * RPC:(i + 1) * RPC]
        xt_shards.append(np.ascontiguousarray(sh))

    res = _run_device(xt_shards, W1, W2, b1, b2v, trace=_trace)

    # Gather device outputs.
    e = np.empty(N, dtype=np.float32)
    waccs = []
    for i in range(NCORES):
        eo = res.results[i]["eout"].astype(np.float32)
        # eout[g, p*G + m] holds e for row p*CHUNK + 512g + m: regroup to
        # chunk-major order.
        eo = eo.reshape(4, NCHUNK, G).transpose(1, 0, 2).reshape(-1)
        e[i * RPC:(i + 1) * RPC] = eo[:RPC]
        waccs.append(res.results[i]["wacc"])

    # Denominators: segment sums of e (same bf16 values the device used).
    denom = np.bincount(bi64, weights=e.astype(np.float64), minlength=B)

    # Numerators: pure windows from device sums; boundary windows recomputed.
    num = np.zeros((B, D), dtype=np.float64)
    for i in range(NCORES):
        wacc_i = waccs[i]
        base = i * RPC
        for w in range(NWIN):
            glo = base + w * WIN
            if glo >= base + RPC:
                break
            ghi = min(glo + WIN, base + RPC)
            b_first = bi64[glo]
            b_last = bi64[ghi - 1]
            if b_first == b_last:
                num[b_first] += wacc_i[:, w]
            else:
                sub = bi64[glo:ghi]
                cuts = np.flatnonzero(np.diff(sub)) + 1
                bounds = np.concatenate(([0], cuts, [ghi - glo]))
                for k in range(len(bounds) - 1):
                    lo, hi = glo + bounds[k], glo + bounds[k + 1]
                    num[sub[bounds[k]]] += \
                        e[lo:hi].astype(np.float64) @ x[lo:hi].astype(np.float64)

    dn = denom[:, None]
    out = np.divide(num, dn, out=np.zeros_like(num), where=dn > 0)
    out = out.astype(np.float32)
    if _want_results:
        return out, res
    return out

